# revision 1
# baseline (speedup 1.0000x reference)
"""VQ codebook pairwise squared-euclidean distances on 8 trn2 NeuronCores.

out[n, u] = ||x_n||^2 + ||w_u||^2 - 2 * x_n . w_u
  inputs: [16384, 1024] f32, w: [4096, 1024] f32 -> out [16384, 4096] f32

Strategy: data-parallel shard of N across 8 cores (2048 rows each), W
replicated. Per core: bf16 GEMM on the tensor engine (fp32 PSUM accum),
epilogue fuses the two rank-1 terms on ScalarE (per-partition bias
-2*psum + x_sq) and VectorE (+w_sq broadcast tile). Host preps bf16
transposed layouts (K-major) so no on-device transposes are needed.
"""

import sys

import ml_dtypes
import numpy as np

if "/opt/trn_rl_repo" not in sys.path:
    sys.path.insert(0, "/opt/trn_rl_repo")

N, D, U = 16384, 1024, 4096
NCORES = 8
NS = N // NCORES  # 2048 rows per core
P = 128
KT = D // P  # 8 k-tiles
MT = NS // P  # 16 m-tiles per core
UT = U // 512  # 8 u-tiles of 512 cols
MC = 4  # m-tiles per xt load chunk (512 cols)

_cache = {}


def _build():
    import concourse.bacc as bacc
    import concourse.mybir as mybir
    import concourse.tile as tile

    dt = mybir.dt
    AF = mybir.ActivationFunctionType
    ALU = mybir.AluOpType

    nc = bacc.Bacc("TRN2", debug=False, target_bir_lowering=False)
    xt_d = nc.dram_tensor("xt", [D, NS], dt.bfloat16, kind="ExternalInput")
    wt_d = nc.dram_tensor("wt", [D, U], dt.bfloat16, kind="ExternalInput")
    xsq_d = nc.dram_tensor("xsq", [P, MT], dt.float32, kind="ExternalInput")
    wsq_d = nc.dram_tensor("wsq", [P, U], dt.float32, kind="ExternalInput")
    out_d = nc.dram_tensor("out", [NS, U], dt.float32, kind="ExternalOutput")

    with tile.TileContext(nc) as tc:
        with (
            tc.tile_pool(name="const", bufs=1) as cpool,
            tc.tile_pool(name="psum", bufs=4, space="PSUM") as psum_pool,
            tc.tile_pool(name="outp", bufs=8) as out_pool,
        ):
            xsq_sb = cpool.tile([P, MT], dt.float32, tag="xsq")
            nc.sync.dma_start(xsq_sb[:], xsq_d[:, :])
            wsq_sb = cpool.tile([P, U], dt.float32, tag="wsq")
            nc.sync.dma_start(wsq_sb[:], wsq_d[:, :])

            xt_sb = {}
            wt_sb = {}

            def load_xt(mc):
                for k in range(KT):
                    t = cpool.tile([P, P * MC], dt.bfloat16, tag=f"xt_{k}_{mc}")
                    nc.sync.dma_start(
                        t[:], xt_d[k * P : (k + 1) * P, mc * P * MC : (mc + 1) * P * MC]
                    )
                    xt_sb[(k, mc)] = t

            def load_wt(u):
                for k in range(KT):
                    t = cpool.tile([P, 512], dt.bfloat16, tag=f"wt_{k}_{u}")
                    nc.sync.dma_start(
                        t[:], wt_d[k * P : (k + 1) * P, u * 512 : (u + 1) * 512]
                    )
                    wt_sb[(k, u)] = t

            # DMA program order = consumption priority: first compute tile
            # needs xt chunk 0 + wt col-block 0; remaining chunks stream in
            # behind compute.
            load_xt(0)
            load_wt(0)
            load_xt(1)
            load_xt(2)
            load_xt(3)
            for u in range(1, UT):
                load_wt(u)

            for u in range(UT):
                for m in range(MT):
                    mc, mo = divmod(m, MC)
                    ps = psum_pool.tile([P, 512], dt.float32, tag="ps")
                    for k in range(KT):
                        nc.tensor.matmul(
                            ps[:],
                            xt_sb[(k, mc)][:, mo * P : (mo + 1) * P],
                            wt_sb[(k, u)][:],
                            start=(k == 0),
                            stop=(k == KT - 1),
                        )
                    ot = out_pool.tile([P, 512], dt.float32, tag="ot")
                    nc.scalar.activation(
                        ot[:], ps[:], AF.Identity, bias=xsq_sb[:, m : m + 1], scale=-2.0
                    )
                    nc.vector.tensor_tensor(
                        ot[:], ot[:], wsq_sb[:, u * 512 : (u + 1) * 512], ALU.add
                    )
                    nc.sync.dma_start(
                        out_d[m * P : (m + 1) * P, u * 512 : (u + 1) * 512], ot[:]
                    )
    nc.compile()
    return nc


def _get_nc():
    if "nc" not in _cache:
        _cache["nc"] = _build()
    return _cache["nc"]


def _prep_inputs(inputs, w):
    bf16 = ml_dtypes.bfloat16
    x = np.ascontiguousarray(np.asarray(inputs, dtype=np.float32))
    wf = np.ascontiguousarray(np.asarray(w, dtype=np.float32))

    wt = np.ascontiguousarray(wf.astype(bf16).T)  # [D, U] bf16
    w_sq = (wf.astype(np.float64) ** 2).sum(-1).astype(np.float32)  # [U]
    wsq_bc = np.ascontiguousarray(np.broadcast_to(w_sq[None, :], (P, U)))
    x_sq = (x.astype(np.float64) ** 2).sum(-1).astype(np.float32)  # [N]

    in_maps = []
    for c in range(NCORES):
        xs = x[c * NS : (c + 1) * NS]
        xt = np.ascontiguousarray(xs.astype(bf16).T)  # [D, NS] bf16
        xsq_t = np.ascontiguousarray(
            x_sq[c * NS : (c + 1) * NS].reshape(MT, P).T
        )  # [P, MT]
        in_maps.append({"xt": xt, "wt": wt, "xsq": xsq_t, "wsq": wsq_bc})
    return in_maps


def run(inputs, w, trace=False, **trace_kwargs):
    """Run on hardware; returns (out, BassKernelResults)."""
    from concourse.bass_utils import run_bass_kernel_spmd

    nc = _get_nc()
    in_maps = _prep_inputs(inputs, w)
    res = run_bass_kernel_spmd(
        nc, in_maps, list(range(NCORES)), trace=trace, **trace_kwargs
    )
    out = np.concatenate([r["out"] for r in res.results], axis=0)
    return np.ascontiguousarray(out, dtype=np.float32), res


def kernel(inputs, w):
    out, _ = run(inputs, w)
    return out


# revision 2
# speedup vs baseline: 1.1384x; 1.1384x over previous
"""VQ codebook pairwise squared-euclidean distances on 8 trn2 NeuronCores.

out[n, u] = ||x_n||^2 + ||w_u||^2 - 2 * x_n . w_u
  inputs: [16384, 1024] f32, w: [4096, 1024] f32 -> out [16384, 4096] f32

Strategy: data-parallel shard of N across 8 cores (2048 rows each), W
replicated. Per core: bf16 GEMM on the tensor engine (fp32 PSUM accum),
epilogue fuses the two rank-1 terms on ScalarE (per-partition bias
-2*psum + x_sq) and VectorE (+w_sq broadcast tile). Host preps bf16
transposed layouts (K-major) so no on-device transposes are needed.
"""

import sys

import ml_dtypes
import numpy as np

if "/opt/trn_rl_repo" not in sys.path:
    sys.path.insert(0, "/opt/trn_rl_repo")

N, D, U = 16384, 1024, 4096
NCORES = 8
NS = N // NCORES  # 2048 rows per core
P = 128
KT = D // P  # 8 k-tiles
MT = NS // P  # 16 m-tiles per core
UT = U // 512  # 8 u-tiles of 512 cols
MC = 4  # m-tiles per xt load chunk (512 cols)

_cache = {}


def _build():
    import concourse.bacc as bacc
    import concourse.mybir as mybir
    import concourse.tile as tile

    dt = mybir.dt
    AF = mybir.ActivationFunctionType
    ALU = mybir.AluOpType

    nc = bacc.Bacc("TRN2", debug=False, target_bir_lowering=False)
    xt_d = nc.dram_tensor("xt", [D, NS], dt.bfloat16, kind="ExternalInput")
    wt_d = nc.dram_tensor("wt", [D, U], dt.bfloat16, kind="ExternalInput")
    xsq_d = nc.dram_tensor("xsq", [P, MT], dt.float32, kind="ExternalInput")
    wsq_d = nc.dram_tensor("wsq", [P, U], dt.float32, kind="ExternalInput")
    out_d = nc.dram_tensor("out", [NS, U], dt.float32, kind="ExternalOutput")

    # K-major views: [p=128, k, cols] so one DMA pulls all 8 k-tiles of a
    # 512-col block (1 MB strided transfer, 1 KB contiguous runs).
    xt_v = xt_d.rearrange("(k p) n -> p k n", p=P)
    wt_v = wt_d.rearrange("(k p) u -> p k u", p=P)

    with tile.TileContext(nc) as tc:
        with (
            tc.tile_pool(name="const", bufs=1) as cpool,
            tc.tile_pool(name="psum", bufs=4, space="PSUM") as psum_pool,
            tc.tile_pool(name="outp", bufs=8) as out_pool,
        ):
            # Small epilogue constants ride the scalar HWDGE ring, which is
            # otherwise idle until outputs start.
            xsq_sb = cpool.tile([P, MT], dt.float32, tag="xsq")
            nc.scalar.dma_start(xsq_sb[:], xsq_d[:, :])
            wsq_sb = cpool.tile([P, U], dt.float32, tag="wsq")
            nc.scalar.dma_start(wsq_sb[:], wsq_d[:, :])

            xt_sb = {}
            wt_sb = {}

            def load_xt(mc):
                t = cpool.tile([P, KT, 512], dt.bfloat16, tag=f"xt_{mc}")
                nc.sync.dma_start(t[:], xt_v[:, :, mc * 512 : (mc + 1) * 512])
                xt_sb[mc] = t

            def load_wt(u):
                t = cpool.tile([P, KT, 512], dt.bfloat16, tag=f"wt_{u}")
                nc.sync.dma_start(t[:], wt_v[:, :, u * 512 : (u + 1) * 512])
                wt_sb[u] = t

            # DMA program order = consumption priority on the sync ring.
            load_wt(0)
            load_xt(0)
            load_xt(1)
            load_xt(2)
            load_xt(3)
            for u in range(1, UT):
                load_wt(u)

            for u in range(UT):
                for m in range(MT):
                    mc, mo = divmod(m, MC)
                    ps = psum_pool.tile([P, 512], dt.float32, tag="ps")
                    for k in range(KT):
                        nc.tensor.matmul(
                            ps[:],
                            xt_sb[mc][:, k, mo * P : (mo + 1) * P],
                            wt_sb[u][:, k, :],
                            start=(k == 0),
                            stop=(k == KT - 1),
                        )
                    ot = out_pool.tile([P, 512], dt.float32, tag="ot")
                    nc.scalar.activation(
                        ot[:], ps[:], AF.Identity, bias=xsq_sb[:, m : m + 1], scale=-2.0
                    )
                    nc.vector.tensor_tensor(
                        ot[:], ot[:], wsq_sb[:, u * 512 : (u + 1) * 512], ALU.add
                    )
                    # Outputs go out on the scalar HWDGE ring so they don't
                    # FIFO behind the input stream on the sync ring.
                    nc.scalar.dma_start(
                        out_d[m * P : (m + 1) * P, u * 512 : (u + 1) * 512], ot[:]
                    )
    nc.compile()
    return nc


def _get_nc():
    if "nc" not in _cache:
        _cache["nc"] = _build()
    return _cache["nc"]


def _prep_inputs(inputs, w):
    bf16 = ml_dtypes.bfloat16
    x = np.ascontiguousarray(np.asarray(inputs, dtype=np.float32))
    wf = np.ascontiguousarray(np.asarray(w, dtype=np.float32))

    wt = np.ascontiguousarray(wf.astype(bf16).T)  # [D, U] bf16
    w_sq = (wf.astype(np.float64) ** 2).sum(-1).astype(np.float32)  # [U]
    wsq_bc = np.ascontiguousarray(np.broadcast_to(w_sq[None, :], (P, U)))
    x_sq = (x.astype(np.float64) ** 2).sum(-1).astype(np.float32)  # [N]

    in_maps = []
    for c in range(NCORES):
        xs = x[c * NS : (c + 1) * NS]
        xt = np.ascontiguousarray(xs.astype(bf16).T)  # [D, NS] bf16
        xsq_t = np.ascontiguousarray(
            x_sq[c * NS : (c + 1) * NS].reshape(MT, P).T
        )  # [P, MT]
        in_maps.append({"xt": xt, "wt": wt, "xsq": xsq_t, "wsq": wsq_bc})
    return in_maps


def run(inputs, w, trace=False, **trace_kwargs):
    """Run on hardware; returns (out, BassKernelResults)."""
    from concourse.bass_utils import run_bass_kernel_spmd

    nc = _get_nc()
    in_maps = _prep_inputs(inputs, w)
    res = run_bass_kernel_spmd(
        nc, in_maps, list(range(NCORES)), trace=trace, **trace_kwargs
    )
    out = np.concatenate([r["out"] for r in res.results], axis=0)
    return np.ascontiguousarray(out, dtype=np.float32), res


def kernel(inputs, w):
    out, _ = run(inputs, w)
    return out
